# revision 9
# baseline (speedup 1.0000x reference)
# Trainium2 Bass kernel for nn_DEERLIFNode (DEER fixed-point LIF neuron).
#
# Math (from the reference, with TAU=2, VTH=0.7, VRESET=0, ALPHA=4):
#   warmstart: y[0] = 0.5*(x[0]+v0); y[1] = 0.5*(x[1]+y[0]); y[2:] = 0
#   repeat NITER times:
#     ys[t] = y[t-1] (ys[0] = v0)
#     t1    = x + ys
#     s     = sigmoid(2*t1 - 2.8)
#     a     = 0.5*(1 - 4*s*(1-s)) = 2*(s-0.5)^2 = 0.5*tanh(t1 - 1.4)^2
#     b     = 0.5*t1 - a*ys
#     y[t]  = a[t]*y[t-1] + b[t]        (linear scan, y[-1] = v0)
#   spike = (y >= 0.7)
#
# The clip(h, +-20) in the reference only matters where sigmoid saturates to
# exactly 0/1 in fp32 either way, so it is dropped.
#
# Layout: lanes = (b, f) pairs on SBUF partitions, time on the free axis.
# Each of the 8 cores takes 2048 lanes = 16 partition-tiles of [128, 1024].
# All work is per-lane, so there is no cross-core communication.  The time
# recurrence is one hardware tensor_tensor_scan per tile per iteration.
#
# Engine split per iteration (per [128,1024] tile):
#   GPSIMD(Pool): t1 = x + ys               (tensor_add)
#   ACT         : v = tanh(t1 - 1.4);  a = square(v/sqrt(2))
#   DVE         : t5 = a*ys; y' = scan(a, b, v0)
#   PE          : b = (0.5 I)@t1 + (-I)@t5   (PSUM accumulate)
#
# The PE runs b for EVERY iteration: the first PE_ITERS iterations feed it
# float32r inputs (1 cycle/row), later iterations feed full fp32 (4
# cycles/row).  fp32 identity matmul is exact (weights 1.0/0.5/-1.0 are
# exact in any mantissa width and PSUM accumulates in fp32), so the late
# iterations lose nothing.
#
# float32r has a 12-bit mantissa (measured: ~2.4e-4 relative rounding), but
# the DEER iteration contracts perturbations ~3.3x per iteration, so using
# the PE for the first PE_ITERS iterations leaves a ~1e-6 error in y
# (numerically: 3 spike flips out of 16.7M at PE_ITERS=6).
#
# Tiles are emitted in interleaved groups of G=4 so each engine's (fixed)
# instruction order alternates between independent tiles instead of stalling
# on the cross-engine dependency chain of a single tile.

import os
import sys

for _p in ("/root/.axon_site/_ro/trn_rl_repo", "/opt/trn_rl_repo"):
    if os.path.isdir(_p) and _p not in sys.path:
        sys.path.insert(0, _p)

from contextlib import ExitStack

import numpy as np

import concourse.bass as bass
import concourse.tile as tile
from concourse import bacc, mybir
from concourse.bass_utils import run_bass_kernel_spmd

T, B, F = 1024, 32, 512
NCORES = 8
LANES = B * F          # 16384
LPC = LANES // NCORES  # 2048 lanes per core
P = 128
NTILES = LPC // P      # 16 tiles per core
NITER = 10
PE_ITERS = 6           # iterations whose b runs on the PE in float32r
G = 8                  # tiles interleaved per group
VTH = 0.7
ISQRT2 = float(1.0 / np.sqrt(2.0))

f32 = mybir.dt.float32
f32r = mybir.dt.float32r
AFT = mybir.ActivationFunctionType
OP = mybir.AluOpType


def _body(ctx, tc, nc, x_d, v0_d, w_d, y_d, s_d, pe_iters, niter, G):
    cpool = ctx.enter_context(tc.tile_pool(name="const", bufs=1))
    xp = ctx.enter_context(tc.tile_pool(name="xp", bufs=G + 1))
    yp = ctx.enter_context(tc.tile_pool(name="yp", bufs=2 * G + 2))
    t1p = ctx.enter_context(tc.tile_pool(name="t1p", bufs=min(G + 1, 6)))
    apool = ctx.enter_context(tc.tile_pool(name="apool", bufs=min(G + 1, 6)))
    vp = ctx.enter_context(tc.tile_pool(name="vp", bufs=3))
    t5p = ctx.enter_context(tc.tile_pool(name="t5p", bufs=min(G, 5)))
    smallp = ctx.enter_context(tc.tile_pool(name="smallp", bufs=4))
    spkp = ctx.enter_context(tc.tile_pool(name="spkp", bufs=2))
    bps = ctx.enter_context(tc.tile_pool(name="bps", bufs=4, space="PSUM"))

    v0t = cpool.tile([P, NTILES], f32)
    nc.sync.dma_start(v0t[:], v0_d[:])
    bm14 = cpool.tile([P, 1], f32)
    nc.vector.memset(bm14[:], -1.4)
    halfs = cpool.tile([P, 2], f32)
    nc.vector.memset(halfs[:], 0.5)
    wt = cpool.tile([P, 256], f32)
    nc.sync.dma_start(wt[:], w_d[:])
    wr = cpool.tile([P, 256], f32r)
    nc.vector.tensor_copy(wr[:], wt[:])

    groups = [list(range(s, min(s + G, NTILES))) for s in range(0, NTILES, G)]
    for group in groups:
        tiles = []
        for i in group:
            rows = slice(i * P, (i + 1) * P)
            v0c = v0t[:, i : i + 1]

            xt = xp.tile([P, T], f32, tag="x")
            nc.sync.dma_start(xt[:], x_d[rows, :])

            # ypad[:, 0] = v0; ypad[:, 1:T+1] = y.  ys == ypad[:, 0:T].
            ya = yp.tile([P, T + 1], f32, tag="ypad")
            yb = yp.tile([P, T + 1], f32, tag="ypad")

            # warmstart: y[0:2] via a tiny scan with a=0.5, b=0.5*x[0:2]
            xh2 = smallp.tile([P, 2], f32, tag="xh2")
            nc.vector.tensor_scalar_mul(xh2[:], xt[:, 0:2], 0.5)
            nc.vector.tensor_tensor_scan(
                ya[:, 1:3], halfs[:], xh2[:], v0c, OP.mult, OP.add
            )
            nc.scalar.memzero(ya[:, 3 : T + 1])
            nc.scalar.copy(ya[:, 0:1], v0c)
            nc.scalar.copy(yb[:, 0:1], v0c)
            tiles.append({"rows": rows, "v0c": v0c, "x": xt, "cur": ya, "nxt": yb})

        for it in range(niter):
            on_pe = it < pe_iters
            dt_i = f32r if on_pe else f32
            for tl in tiles:
                ys = tl["cur"][:, 0:T]
                t1 = t1p.tile([P, T], dt_i, tag="t1")
                nc.gpsimd.tensor_add(t1[:], tl["x"][:], ys)
                t1f = t1[:].bitcast(f32) if on_pe else t1[:]
                v = vp.tile([P, T], f32, tag="v")
                nc.scalar.activation(v[:], t1f, AFT.Tanh, bias=bm14[:], scale=1.0)
                a = apool.tile([P, T], f32, tag="a")
                nc.scalar.activation(a[:], v[:], AFT.Square, bias=0.0, scale=ISQRT2)
                b = bps.tile([P, T], f32, tag="b")
                halves = [slice(0, 512), slice(512, 1024)]
                if on_pe:
                    # b = 0.5*t1 - t5,  y' = scan(a, b, v0)
                    t5 = t5p.tile([P, T], dt_i, tag="t5")
                    nc.vector.tensor_mul(t5[:], a[:], ys)
                    for c in halves:  # same weights back-to-back: fewer loads
                        nc.tensor.matmul(
                            b[:, c], wr[:, 0:128], t1[:, c], start=True, stop=False
                        )
                    for c in halves:
                        nc.tensor.matmul(
                            b[:, c], wr[:, 128:256], t5[:, c], start=False, stop=True
                        )
                    nc.vector.tensor_tensor_scan(
                        tl["nxt"][:, 1 : T + 1], a[:], b[:], tl["v0c"],
                        OP.mult, OP.add,
                    )
                else:
                    # residual form: r = 0.5*t1 - y (aligned), w = scan(a, r, 0),
                    # y' = y + w.  No t5; r's PE inputs don't pass through ACT.
                    yold = tl["cur"][:, 1 : T + 1]
                    for c in halves:
                        nc.tensor.matmul(
                            b[:, c], wt[:, 0:128], t1[:, c], start=True, stop=False
                        )
                    for c in halves:
                        nc.tensor.matmul(
                            b[:, c],
                            wt[:, 128:256],
                            tl["cur"][:, 1 + c.start : 1 + c.stop],
                            start=False,
                            stop=True,
                        )
                    wsc = t5p.tile([P, T], f32, tag="t5")
                    nc.vector.tensor_tensor_scan(
                        wsc[:], a[:], b[:], 0.0, OP.mult, OP.add
                    )
                    nc.vector.tensor_add(tl["nxt"][:, 1 : T + 1], yold, wsc[:])
                tl["cur"], tl["nxt"] = tl["nxt"], tl["cur"]

        for tl in tiles:
            yfin = tl["cur"][:, 1 : T + 1]
            spk = spkp.tile([P, T], f32, tag="spk")
            nc.gpsimd.tensor_scalar(spk[:], yfin, VTH, None, OP.is_ge)
            nc.sync.dma_start(y_d[tl["rows"], :], yfin)
            nc.sync.dma_start(s_d[tl["rows"], :], spk[:])


def _build(pe_iters=PE_ITERS, niter=NITER, G=G):
    nc = bacc.Bacc("TRN2", target_bir_lowering=False, debug=False, num_devices=NCORES)
    x_d = nc.declare_dram_parameter("x", [LPC, T], f32, isOutput=False)
    v0_d = nc.declare_dram_parameter("v0", [P, NTILES], f32, isOutput=False)
    w_d = nc.declare_dram_parameter("w", [P, 256], f32, isOutput=False)
    y_d = nc.declare_dram_parameter("y", [LPC, T], f32, isOutput=True)
    s_d = nc.declare_dram_parameter("spk", [LPC, T], f32, isOutput=True)

    with tile.TileContext(nc) as tc:
        with ExitStack() as ctx:
            _body(ctx, tc, nc, x_d.ap(), v0_d.ap(), w_d.ap(), y_d.ap(), s_d.ap(),
                  pe_iters, niter, G)
    nc.compile()
    return nc


_NC_CACHE = {}


def _get_nc(pe_iters=PE_ITERS, niter=NITER, G_=None):
    key = (pe_iters, niter, G_ or G)
    if key not in _NC_CACHE:
        _NC_CACHE[key] = _build(pe_iters, niter, G_ or G)
    return _NC_CACHE[key]


def _make_in_maps(x, v_init):
    x = np.ascontiguousarray(np.asarray(x, dtype=np.float32))
    v = np.ascontiguousarray(np.asarray(v_init, dtype=np.float32))
    assert x.shape == (T, B, F), x.shape
    assert v.shape == (B, F), v.shape
    xt = np.ascontiguousarray(x.reshape(T, LANES).T)  # (LANES, T)
    vf = v.reshape(LANES)
    w = np.concatenate(
        [0.5 * np.eye(P, dtype=np.float32), -np.eye(P, dtype=np.float32)], axis=1
    )
    in_maps = []
    for k in range(NCORES):
        sl = slice(k * LPC, (k + 1) * LPC)
        in_maps.append(
            {
                "x": np.ascontiguousarray(xt[sl]),
                "v0": np.ascontiguousarray(vf[sl].reshape(NTILES, P).T),
                "w": w,
            }
        )
    return in_maps


def _assemble(results):
    y = np.concatenate([r["y"] for r in results], axis=0)  # (LANES, T)
    s = np.concatenate([r["spk"] for r in results], axis=0)
    y_full = np.ascontiguousarray(y.T).reshape(T, B, F)
    s_full = np.ascontiguousarray(s.T).reshape(T, B, F)
    return s_full, y_full


def run(x, v_init, pe_iters=PE_ITERS, niter=NITER, trace=False, G_=None, **kw):
    nc = _get_nc(pe_iters, niter, G_)
    in_maps = _make_in_maps(x, v_init)
    res = run_bass_kernel_spmd(
        nc, in_maps, core_ids=list(range(NCORES)), trace=trace, **kw
    )
    spike, y = _assemble(res.results)
    return spike, y, res


def kernel(x, v_init):
    spike, y, _ = run(x, v_init)
    return spike, y


# revision 10
# speedup vs baseline: 1.0101x; 1.0101x over previous
# Trainium2 Bass kernel for nn_DEERLIFNode (DEER fixed-point LIF neuron).
#
# Math (from the reference, with TAU=2, VTH=0.7, VRESET=0, ALPHA=4):
#   warmstart: y[0] = 0.5*(x[0]+v0); y[1] = 0.5*(x[1]+y[0]); y[2:] = 0
#   repeat NITER times:
#     ys[t] = y[t-1] (ys[0] = v0)
#     t1    = x + ys
#     s     = sigmoid(2*t1 - 2.8)
#     a     = 0.5*(1 - 4*s*(1-s)) = 2*(s-0.5)^2 = 0.5*tanh(t1 - 1.4)^2
#     b     = 0.5*t1 - a*ys
#     y[t]  = a[t]*y[t-1] + b[t]        (linear scan, y[-1] = v0)
#   spike = (y >= 0.7)
#
# The clip(h, +-20) in the reference only matters where sigmoid saturates to
# exactly 0/1 in fp32 either way, so it is dropped.
#
# Layout: lanes = (b, f) pairs on SBUF partitions, time on the free axis.
# Each of the 8 cores takes 2048 lanes = 16 partition-tiles of [128, 1024].
# All work is per-lane, so there is no cross-core communication.  The time
# recurrence is one hardware tensor_tensor_scan per tile per iteration.
#
# Engine split per iteration (per [128,1024] tile):
#   GPSIMD(Pool): t1 = x + ys               (tensor_add)
#   ACT         : v = tanh(t1 - 1.4);  a = square(v/sqrt(2))
#   DVE         : t5 = a*ys; y' = scan(a, b, v0)
#   PE          : b = (0.5 I)@t1 + (-I)@t5   (PSUM accumulate)
#
# The PE runs b for EVERY iteration: the first PE_ITERS iterations feed it
# float32r inputs (1 cycle/row), later iterations feed full fp32 (4
# cycles/row).  fp32 identity matmul is exact (weights 1.0/0.5/-1.0 are
# exact in any mantissa width and PSUM accumulates in fp32), so the late
# iterations lose nothing.
#
# float32r has a 12-bit mantissa (measured: ~2.4e-4 relative rounding), but
# the DEER iteration contracts perturbations ~3.3x per iteration, so using
# the PE for the first PE_ITERS iterations leaves a ~1e-6 error in y
# (numerically: 3 spike flips out of 16.7M at PE_ITERS=6).
#
# Tiles are emitted in interleaved groups of G=4 so each engine's (fixed)
# instruction order alternates between independent tiles instead of stalling
# on the cross-engine dependency chain of a single tile.

import os
import sys

for _p in ("/root/.axon_site/_ro/trn_rl_repo", "/opt/trn_rl_repo"):
    if os.path.isdir(_p) and _p not in sys.path:
        sys.path.insert(0, _p)

from contextlib import ExitStack

import numpy as np

import concourse.bass as bass
import concourse.tile as tile
from concourse import bacc, mybir
from concourse.bass_utils import run_bass_kernel_spmd

T, B, F = 1024, 32, 512
NCORES = 8
LANES = B * F          # 16384
LPC = LANES // NCORES  # 2048 lanes per core
P = 128
NTILES = LPC // P      # 16 tiles per core
NITER = 10
PE_ITERS = 6           # iterations whose b runs on the PE in float32r
G = 8                  # tiles interleaved per group
VTH = 0.7
ISQRT2 = float(1.0 / np.sqrt(2.0))

f32 = mybir.dt.float32
f32r = mybir.dt.float32r
AFT = mybir.ActivationFunctionType
OP = mybir.AluOpType


def _body(ctx, tc, nc, x_d, v0_d, w_d, y_d, s_d, pe_iters, niter, G):
    cpool = ctx.enter_context(tc.tile_pool(name="const", bufs=1))
    xp = ctx.enter_context(tc.tile_pool(name="xp", bufs=G + 1))
    yp = ctx.enter_context(tc.tile_pool(name="yp", bufs=2 * G + 2))
    t1p = ctx.enter_context(tc.tile_pool(name="t1p", bufs=min(G + 1, 6)))
    apool = ctx.enter_context(tc.tile_pool(name="apool", bufs=min(G + 1, 6)))
    vp = ctx.enter_context(tc.tile_pool(name="vp", bufs=3))
    t5p = ctx.enter_context(tc.tile_pool(name="t5p", bufs=min(G, 5)))
    smallp = ctx.enter_context(tc.tile_pool(name="smallp", bufs=4))
    spkp = ctx.enter_context(tc.tile_pool(name="spkp", bufs=2))
    bps = ctx.enter_context(tc.tile_pool(name="bps", bufs=4, space="PSUM"))

    v0t = cpool.tile([P, NTILES], f32)
    nc.sync.dma_start(v0t[:], v0_d[:])
    bm14 = cpool.tile([P, 1], f32)
    nc.vector.memset(bm14[:], -1.4)
    halfs = cpool.tile([P, 2], f32)
    nc.vector.memset(halfs[:], 0.5)
    wt = cpool.tile([P, 256], f32)
    nc.sync.dma_start(wt[:], w_d[:])
    wr = cpool.tile([P, 256], f32r)
    nc.vector.tensor_copy(wr[:], wt[:])

    groups = [list(range(s, min(s + G, NTILES))) for s in range(0, NTILES, G)]
    for group in groups:
        tiles = []
        for i in group:
            rows = slice(i * P, (i + 1) * P)
            v0c = v0t[:, i : i + 1]

            xt = xp.tile([P, T], f32, tag="x")
            nc.sync.dma_start(xt[:], x_d[rows, :])

            # ypad[:, 0] = v0; ypad[:, 1:T+1] = y.  ys == ypad[:, 0:T].
            ya = yp.tile([P, T + 1], f32, tag="ypad")
            yb = yp.tile([P, T + 1], f32, tag="ypad")

            # warmstart: y[0:2] via a tiny scan with a=0.5, b=0.5*x[0:2]
            xh2 = smallp.tile([P, 2], f32, tag="xh2")
            nc.vector.tensor_scalar_mul(xh2[:], xt[:, 0:2], 0.5)
            nc.vector.tensor_tensor_scan(
                ya[:, 1:3], halfs[:], xh2[:], v0c, OP.mult, OP.add
            )
            nc.scalar.copy(ya[:, 0:1], v0c)
            nc.scalar.copy(yb[:, 0:1], v0c)
            tiles.append({"rows": rows, "v0c": v0c, "x": xt, "cur": ya, "nxt": yb})

        for it in range(niter):
            on_pe = it < pe_iters
            dt_i = f32r if on_pe else f32
            for tl in tiles:
                halves = [slice(0, 512), slice(512, 1024)]
                if it == 0:
                    # ys = [v0, w0, w1, 0, ..., 0]: t1 == x except cols 0:3,
                    # t5 == 0 except cols 0:3, and ya[:, 3:] is never read so
                    # it needs no memzero.
                    ysb = tl["cur"][:, 0:3]
                    t1b = smallp.tile([P, 3], f32, tag="t1b")
                    nc.gpsimd.tensor_add(t1b[:], tl["x"][:, 0:3], ysb)
                    v = vp.tile([P, T], f32, tag="v")
                    nc.scalar.activation(
                        v[:, 0:3], t1b[:], AFT.Tanh, bias=bm14[:], scale=1.0
                    )
                    nc.scalar.activation(
                        v[:, 3:T], tl["x"][:, 3:T], AFT.Tanh, bias=bm14[:], scale=1.0
                    )
                    a = apool.tile([P, T], f32, tag="a")
                    nc.scalar.activation(
                        a[:], v[:], AFT.Square, bias=0.0, scale=ISQRT2
                    )
                    t5b = smallp.tile([P, 3], f32, tag="t5b")
                    nc.vector.tensor_mul(t5b[:], a[:, 0:3], ysb)
                    b = bps.tile([P, T], f32, tag="b")
                    for c in halves:
                        nc.tensor.matmul(
                            b[:, c], wt[:, 0:128], tl["x"][:, c],
                            start=True, stop=True,
                        )
                    nc.vector.scalar_tensor_tensor(
                        b[:, 0:3], t1b[:], 0.5, t5b[:], OP.mult, OP.subtract
                    )
                    nc.vector.tensor_tensor_scan(
                        tl["nxt"][:, 1 : T + 1], a[:], b[:], tl["v0c"],
                        OP.mult, OP.add,
                    )
                    tl["cur"], tl["nxt"] = tl["nxt"], tl["cur"]
                    continue
                ys = tl["cur"][:, 0:T]
                t1 = t1p.tile([P, T], dt_i, tag="t1")
                nc.gpsimd.tensor_add(t1[:], tl["x"][:], ys)
                t1f = t1[:].bitcast(f32) if on_pe else t1[:]
                v = vp.tile([P, T], f32, tag="v")
                nc.scalar.activation(v[:], t1f, AFT.Tanh, bias=bm14[:], scale=1.0)
                a = apool.tile([P, T], f32, tag="a")
                nc.scalar.activation(a[:], v[:], AFT.Square, bias=0.0, scale=ISQRT2)
                b = bps.tile([P, T], f32, tag="b")
                if on_pe:
                    # b = 0.5*t1 - t5,  y' = scan(a, b, v0)
                    t5 = t5p.tile([P, T], dt_i, tag="t5")
                    nc.vector.tensor_mul(t5[:], a[:], ys)
                    for c in halves:  # same weights back-to-back: fewer loads
                        nc.tensor.matmul(
                            b[:, c], wr[:, 0:128], t1[:, c], start=True, stop=False
                        )
                    for c in halves:
                        nc.tensor.matmul(
                            b[:, c], wr[:, 128:256], t5[:, c], start=False, stop=True
                        )
                    nc.vector.tensor_tensor_scan(
                        tl["nxt"][:, 1 : T + 1], a[:], b[:], tl["v0c"],
                        OP.mult, OP.add,
                    )
                else:
                    # residual form: r = 0.5*t1 - y (aligned), w = scan(a, r, 0),
                    # y' = y + w.  No t5; r's PE inputs don't pass through ACT.
                    yold = tl["cur"][:, 1 : T + 1]
                    for c in halves:
                        nc.tensor.matmul(
                            b[:, c], wt[:, 0:128], t1[:, c], start=True, stop=False
                        )
                    for c in halves:
                        nc.tensor.matmul(
                            b[:, c],
                            wt[:, 128:256],
                            tl["cur"][:, 1 + c.start : 1 + c.stop],
                            start=False,
                            stop=True,
                        )
                    wsc = t5p.tile([P, T], f32, tag="t5")
                    nc.vector.tensor_tensor_scan(
                        wsc[:], a[:], b[:], 0.0, OP.mult, OP.add
                    )
                    nc.vector.tensor_add(tl["nxt"][:, 1 : T + 1], yold, wsc[:])
                tl["cur"], tl["nxt"] = tl["nxt"], tl["cur"]

        for tl in tiles:
            yfin = tl["cur"][:, 1 : T + 1]
            spk = spkp.tile([P, T], f32, tag="spk")
            nc.gpsimd.tensor_scalar(spk[:], yfin, VTH, None, OP.is_ge)
            nc.sync.dma_start(y_d[tl["rows"], :], yfin)
            nc.sync.dma_start(s_d[tl["rows"], :], spk[:])


def _build(pe_iters=PE_ITERS, niter=NITER, G=G):
    nc = bacc.Bacc("TRN2", target_bir_lowering=False, debug=False, num_devices=NCORES)
    x_d = nc.declare_dram_parameter("x", [LPC, T], f32, isOutput=False)
    v0_d = nc.declare_dram_parameter("v0", [P, NTILES], f32, isOutput=False)
    w_d = nc.declare_dram_parameter("w", [P, 256], f32, isOutput=False)
    y_d = nc.declare_dram_parameter("y", [LPC, T], f32, isOutput=True)
    s_d = nc.declare_dram_parameter("spk", [LPC, T], f32, isOutput=True)

    with tile.TileContext(nc) as tc:
        with ExitStack() as ctx:
            _body(ctx, tc, nc, x_d.ap(), v0_d.ap(), w_d.ap(), y_d.ap(), s_d.ap(),
                  pe_iters, niter, G)
    nc.compile()
    return nc


_NC_CACHE = {}


def _get_nc(pe_iters=PE_ITERS, niter=NITER, G_=None):
    key = (pe_iters, niter, G_ or G)
    if key not in _NC_CACHE:
        _NC_CACHE[key] = _build(pe_iters, niter, G_ or G)
    return _NC_CACHE[key]


def _make_in_maps(x, v_init):
    x = np.ascontiguousarray(np.asarray(x, dtype=np.float32))
    v = np.ascontiguousarray(np.asarray(v_init, dtype=np.float32))
    assert x.shape == (T, B, F), x.shape
    assert v.shape == (B, F), v.shape
    xt = np.ascontiguousarray(x.reshape(T, LANES).T)  # (LANES, T)
    vf = v.reshape(LANES)
    w = np.concatenate(
        [0.5 * np.eye(P, dtype=np.float32), -np.eye(P, dtype=np.float32)], axis=1
    )
    in_maps = []
    for k in range(NCORES):
        sl = slice(k * LPC, (k + 1) * LPC)
        in_maps.append(
            {
                "x": np.ascontiguousarray(xt[sl]),
                "v0": np.ascontiguousarray(vf[sl].reshape(NTILES, P).T),
                "w": w,
            }
        )
    return in_maps


def _assemble(results):
    y = np.concatenate([r["y"] for r in results], axis=0)  # (LANES, T)
    s = np.concatenate([r["spk"] for r in results], axis=0)
    y_full = np.ascontiguousarray(y.T).reshape(T, B, F)
    s_full = np.ascontiguousarray(s.T).reshape(T, B, F)
    return s_full, y_full


def run(x, v_init, pe_iters=PE_ITERS, niter=NITER, trace=False, G_=None, **kw):
    nc = _get_nc(pe_iters, niter, G_)
    in_maps = _make_in_maps(x, v_init)
    res = run_bass_kernel_spmd(
        nc, in_maps, core_ids=list(range(NCORES)), trace=trace, **kw
    )
    spike, y = _assemble(res.results)
    return spike, y, res


def kernel(x, v_init):
    spike, y, _ = run(x, v_init)
    return spike, y
